# revision 17
# baseline (speedup 1.0000x reference)
"""Trainium2 Bass kernel for DCKModule (involution / dynamic conv kernel).

Math (per batch image):
  x  = relu(W1 @ guide * bn_scale + bn_bias)        # (64, 9216)
  dk[c,p] (tap k) = W2exp_k @ x                     # broadcast to 256 ch on PE
  out[c,p] = sum_k dk_k[c,p] * fpad[c, p+off_k] + feature[c,p]

Mapping: data-parallel over batch (1 image per NeuronCore, 8 cores).
BN scale folded into W1 host-side; feature map padded + bf16 host-side.

Engine split (derived from the TimelineSim cost model):
  - PE (bf16, 1 cyc/row): W2exp matmuls for all 49 taps, plus identity-
    matmul accumulation of ~26 taps' products into PSUM, plus folding the
    DVE partial accumulator and the residual into the same PSUM banks.
  - Act: converts dk (PSUM fp32) -> dkb (SBUF bf16) for the DVE-mult taps
    (bf16 operands give DVE its 2x '2x_1p' mode), relu on x, final drain
    of PSUM acc -> fp32 output.
  - DVE: bf16 products for the V taps (2x mode), bf16 adds for ~23 taps.
  - GpSimd(Pool): products for the P taps reading dk straight from PSUM.

The emission is software-pipelined: accumulate matmuls lag the W2exp
matmuls by LAG taps so the PE instruction stream never waits (PE p-state
pricing doubles matmul cost if the engine gaps), and the next block's
x-phase is emitted mid-block.

PSUM: dk [128,1024] x2 bufs (4 banks) + acc [128,1024] x2 (ct0/ct1, 4
banks) = 8 banks. The x-matmul (px) borrows a dk buffer.
"""

import numpy as np
import ml_dtypes

import concourse.bass as bass
import concourse.mybir as mybir
import concourse.tile as tile
from concourse import bacc, bass_utils

B, C, H, W = 8, 256, 96, 96
K7, PAD, G, GC, R = 7, 3, 16, 16, 64
HP = H + 2 * PAD          # 102
PIX = H * W               # 9216
BN_EPS = 1e-5
RBLK = 8                  # output rows per pipeline block
NBLK = H // RBLK          # 12
BLKPIX = RBLK * W         # 768
SUB = 384                 # matmul free-dim chunk per PSUM bank (= 4 rows)
NSUB = BLKPIX // SUB      # 2

F32 = mybir.dt.float32
BF16 = mybir.dt.bfloat16
BF = ml_dtypes.bfloat16
TRACE = False

# Per-tap engine assignment (tuned against TimelineSim).
# mult: 'V' = DVE bf16 (via Act conversion), 'P' = gpsimd from PSUM fp32,
#       'F' = DVE fp32 directly from PSUM (no Act conversion).
# add:  'E' = PE identity-matmul accumulate, 'D' = DVE bf16 add.
import os

_NF = int(os.environ.get("KNF", "0"))     # F-taps (DVE fp32 direct)
_NC = int(os.environ.get("KNC", "6"))     # C-taps (gpsimd conv + DVE mult)
_ND = int(os.environ.get("KND", "27"))    # D-adds (DVE bf16 add)
_NP = int(os.environ.get("KNP", "13"))    # P-taps (gpsimd mult)


def _spread(n, total=49, avoid=()):
    """n tap indices spread evenly over [0,49), avoiding `avoid`."""
    out = []
    cands = sorted(set(range(total)) - set(avoid))
    for i in range(n):
        out.append(cands[int(i * len(cands) / n)])
    return set(out)


_P_SET = _spread(_NP)
_F_SET = _spread(_NF, avoid=_P_SET)
_C_SET = _spread(_NC, avoid=_P_SET | _F_SET)
_D_SET = _spread(_ND)
MULT_ENG = ['P' if k in _P_SET else ('F' if k in _F_SET else
            ('C' if k in _C_SET else 'V')) for k in range(49)]
ADD_ENG = ['D' if k in _D_SET else 'E' for k in range(49)]
LAG = 4                   # taps between mult emission and add emission
X_EMIT = 40               # tap at which next block's x-phase is emitted

_CACHE = {}


def _build_nc():
    nc = bacc.Bacc(None, target_bir_lowering=False)
    fm_d = nc.dram_tensor("fm", [C, HP * HP], BF16, kind="ExternalInput")
    gm_d = nc.dram_tensor("gm", [C, PIX], BF16, kind="ExternalInput")
    w1_d = nc.dram_tensor("w1pt", [C, R], BF16, kind="ExternalInput")
    bias_d = nc.dram_tensor("bias", [R, 1], F32, kind="ExternalInput")
    w2_d = nc.dram_tensor("w2et", [R, 49 * C], BF16, kind="ExternalInput")
    id_d = nc.dram_tensor("ident", [128, 128], BF16, kind="ExternalInput")
    out_d = nc.dram_tensor("out", [C, PIX], F32, kind="ExternalOutput")

    first_e_k = min(k for k in range(49) if ADD_ENG[k] == 'E')
    first_d_k = min(k for k in range(49) if ADD_ENG[k] == 'D')

    with tile.TileContext(nc) as tc:
        with tc.tile_pool(name="persist", bufs=1) as persist, \
             tc.tile_pool(name="gpool", bufs=2) as gpool, \
             tc.tile_pool(name="xpool", bufs=2) as xpool, \
             tc.tile_pool(name="dkbpool", bufs=5) as dkbpool, \
             tc.tile_pool(name="prodpool", bufs=14) as prodpool, \
             tc.tile_pool(name="accdpool", bufs=2) as accdpool, \
             tc.tile_pool(name="outpool", bufs=2) as outpool, \
             tc.tile_pool(name="psdk", bufs=4, space="PSUM") as psdk, \
             tc.tile_pool(name="psacc", bufs=2, space="PSUM") as psacc:

            fpad = [persist.tile([128, HP * HP], BF16, tag=f"fpad{ct}",
                                 name=f"fpad{ct}") for ct in range(2)]
            w1_sb = persist.tile([128, 2 * R], BF16, tag="w1", name="w1sb")
            bias_sb = persist.tile([R, 1], F32, tag="bias", name="biassb")
            w2_sb = persist.tile([R, 49 * C], BF16, tag="w2", name="w2sb")
            id_sb = persist.tile([128, 128], BF16, tag="ident", name="idsb")

            for ct in range(2):
                nc.gpsimd.dma_start(
                    out=fpad[ct][:],
                    in_=fm_d[ct * 128:(ct + 1) * 128, :])
            for ck in range(2):
                nc.gpsimd.dma_start(out=w1_sb[:, ck * R:(ck + 1) * R],
                                    in_=w1_d[ck * 128:(ck + 1) * 128, :])
            nc.gpsimd.dma_start(out=bias_sb[:], in_=bias_d[:])
            nc.gpsimd.dma_start(out=w2_sb[:], in_=w2_d[:])
            nc.gpsimd.dma_start(out=id_sb[:], in_=id_d[:])

            fviews = [fpad[ct][:].rearrange("p (r j) -> p r j", j=HP)
                      for ct in range(2)]

            def emit_gm_dma(blk):
                g_sb = [gpool.tile([128, BLKPIX], BF16, tag=f"g{ct}",
                                   name=f"gsb{ct}") for ct in range(2)]
                for ct in range(2):
                    nc.sync.dma_start(
                        out=g_sb[ct][:],
                        in_=gm_d[ct * 128:(ct + 1) * 128,
                                 blk * BLKPIX:(blk + 1) * BLKPIX])
                return g_sb

            def emit_x_phase(g_sb):
                x_sb = xpool.tile([R, BLKPIX], BF16, tag="x", name="xsb")
                for s in range(NSUB):
                    px = psdk.tile([128, 512], F32, tag="dk", name="px")
                    for ck in range(2):
                        nc.tensor.matmul(
                            px[0:R, 0:SUB],
                            w1_sb[:, ck * R:(ck + 1) * R],
                            g_sb[ck][:, s * SUB:(s + 1) * SUB],
                            start=(ck == 0), stop=(ck == 1))
                    nc.vector.tensor_scalar(
                        x_sb[:, s * SUB:(s + 1) * SUB],
                        px[0:R, 0:SUB],
                        bias_sb[:], 0.0,
                        mybir.AluOpType.add, mybir.AluOpType.max)
                return x_sb

            g_sb = emit_gm_dma(0)
            x_cur = emit_x_phase(g_sb)

            # Each (blk, ct) is one pipeline phase over 49 taps. The phase
            # tail (accumulator folds, residual, drain, out-DMA) is deferred
            # into the NEXT phase's tap stream so it overlaps steady-state
            # work instead of draining the pipeline.
            prev_tail = None
            x_next = None

            for phase in range(2 * NBLK + 1):
                blk, ct = phase // 2, phase % 2
                last = phase == 2 * NBLK
                if not last:
                    r0 = blk * RBLK
                    acc = psacc.tile([128, NSUB * 512], F32, tag="acc",
                                     name=f"acc{ct}")
                    accd = accdpool.tile([128, BLKPIX], BF16, tag="accd",
                                         name=f"accd{ct}")
                    pending = []   # (k, prod) awaiting their add emission

                def emit_add(k, prod, acc=None, accd=None, r0=0, ct=0):
                    if ADD_ENG[k] == 'E':
                        for s in range(NSUB):
                            nc.tensor.matmul(
                                acc[:, s * 512:s * 512 + SUB],
                                id_sb[:],
                                prod[:, s * SUB:(s + 1) * SUB],
                                start=(k == first_e_k), stop=False,
                                skip_group_check=True)
                    elif k != first_d_k:
                        nc.vector.tensor_tensor(
                            accd[:], accd[:], prod[:],
                            mybir.AluOpType.add)

                def emit_tail(pending, acc, accd, r0, ct):
                    def run():
                        for k, prod in pending:
                            emit_add(k, prod, acc, accd, r0, ct)
                        for s in range(NSUB):
                            nc.tensor.matmul(
                                acc[:, s * 512:s * 512 + SUB],
                                id_sb[:],
                                accd[:, s * SUB:(s + 1) * SUB],
                                start=False, stop=False,
                                skip_group_check=True)
                        for s in range(NSUB):
                            nc.tensor.matmul(
                                acc[:, s * 512:s * 512 + SUB],
                                id_sb[:],
                                fviews[ct][:, r0 + PAD + 4 * s:
                                           r0 + PAD + 4 * s + 4,
                                           PAD:PAD + W],
                                start=False, stop=(s == NSUB - 1),
                                skip_group_check=True)
                        out_sb = outpool.tile([128, BLKPIX], F32, tag="osb",
                                              name="osb")
                        nc.vector.tensor_copy(
                            out_sb[:].rearrange("p (s q) -> p s q", s=NSUB),
                            acc[:].rearrange("p (s q) -> p s q",
                                             s=NSUB)[:, :, 0:SUB])
                        nc.sync.dma_start(
                            out=out_d[ct * 128:(ct + 1) * 128,
                                      r0 * W:(r0 + RBLK) * W],
                            in_=out_sb[:])
                    return run

                if last:
                    if prev_tail is not None:
                        prev_tail()
                    break

                for k in range(49):
                    di, dj = divmod(k, K7)
                    fsl = fviews[ct][:, r0 + di:r0 + di + RBLK, dj:dj + W]
                    if ADD_ENG[k] == 'D' and k == first_d_k:
                        prod = accd
                    else:
                        prod = prodpool.tile([128, BLKPIX], BF16,
                                             tag="prod", name="prod")

                    dkb = None
                    if MULT_ENG[k] in ('V', 'C'):
                        dkb = dkbpool.tile([128, BLKPIX], BF16,
                                           tag="dkb", name="dkb")
                    for s in range(NSUB):
                        dk = psdk.tile([128, 512], F32, tag="dk", name="dk")
                        nc.tensor.matmul(
                            dk[:, 0:SUB],
                            w2_sb[:, k * C + ct * 128:k * C + ct * 128 + 128],
                            x_cur[:, s * SUB:(s + 1) * SUB],
                            start=True, stop=True)
                        if MULT_ENG[k] == 'V':
                            nc.scalar.activation(
                                dkb[:, s * SUB:(s + 1) * SUB],
                                dk[:, 0:SUB],
                                mybir.ActivationFunctionType.Copy,
                                bias=0.0, scale=1.0)
                        elif MULT_ENG[k] == 'C':
                            nc.gpsimd.tensor_copy(
                                dkb[:, s * SUB:(s + 1) * SUB],
                                dk[:, 0:SUB])
                        elif MULT_ENG[k] == 'P':
                            nc.gpsimd.tensor_tensor(
                                prod[:, s * SUB:(s + 1) * SUB]
                                .rearrange("p (r j) -> p r j", j=W),
                                dk[:, 0:SUB]
                                .rearrange("p (r j) -> p r j", j=W),
                                fviews[ct][:, r0 + di + 4 * s:
                                           r0 + di + 4 * s + 4,
                                           dj:dj + W],
                                mybir.AluOpType.mult)
                        else:   # 'F': DVE fp32 mult straight from PSUM
                            nc.vector.tensor_tensor(
                                prod[:, s * SUB:(s + 1) * SUB]
                                .rearrange("p (r j) -> p r j", j=W),
                                dk[:, 0:SUB]
                                .rearrange("p (r j) -> p r j", j=W),
                                fviews[ct][:, r0 + di + 4 * s:
                                           r0 + di + 4 * s + 4,
                                           dj:dj + W],
                                mybir.AluOpType.mult)

                    if MULT_ENG[k] in ('V', 'C'):
                        nc.vector.tensor_tensor(
                            prod[:].rearrange("p (r j) -> p r j", j=W),
                            dkb[:].rearrange("p (r j) -> p r j", j=W),
                            fsl, mybir.AluOpType.mult)

                    pending.append((k, prod))

                    # previous phase's tail goes early in this phase
                    if k == 1 and prev_tail is not None:
                        prev_tail()
                        prev_tail = None

                    # emit lagged adds (keep the PE stream fed w/ ready work)
                    while pending and pending[0][0] <= k - LAG:
                        kq, pq = pending.pop(0)
                        emit_add(kq, pq, acc, accd, r0, ct)

                    if k == X_EMIT and ct == 1 and blk + 1 < NBLK:
                        g_sb = emit_gm_dma(blk + 1)
                        x_next = emit_x_phase(g_sb)

                prev_tail = emit_tail(list(pending), acc, accd, r0, ct)
                pending = []
                if ct == 1 and blk + 1 < NBLK:
                    x_cur = x_next
    if not nc.is_finalized():
        nc.finalize()
    return nc


def _host_weights(W1, bn_gamma, bn_beta, bn_mean, bn_var, W2):
    inv = bn_gamma / np.sqrt(bn_var + BN_EPS)
    W1p = (W1 * inv[:, None]).astype(np.float32)          # (64, 256)
    w1pt = np.ascontiguousarray(W1p.T).astype(BF)          # (256, 64)
    bias = (bn_beta - bn_mean * inv).astype(np.float32).reshape(R, 1)
    W2r = W2.reshape(G, 49, R)                             # [g, k, o]
    w2et = np.ascontiguousarray(
        np.repeat(W2r.transpose(2, 1, 0)[:, :, :, None], GC, axis=3)
        .reshape(R, 49 * C)).astype(BF)                    # [o, k*256 + c]
    return w1pt, bias, w2et


def kernel(feature_map, guide_map, W1, bn_gamma, bn_beta, bn_mean, bn_var, W2):
    fm4 = np.asarray(feature_map, np.float32).reshape(B, C, H, W)
    fm = np.ascontiguousarray(
        np.pad(fm4, ((0, 0), (0, 0), (PAD, PAD), (PAD, PAD)))
        .reshape(B, C, HP * HP)).astype(BF)
    gm = np.ascontiguousarray(np.asarray(guide_map, np.float32)
                              .reshape(B, C, PIX)).astype(BF)
    w1pt, bias, w2et = _host_weights(
        np.asarray(W1, np.float32), np.asarray(bn_gamma, np.float32),
        np.asarray(bn_beta, np.float32), np.asarray(bn_mean, np.float32),
        np.asarray(bn_var, np.float32), np.asarray(W2, np.float32))
    ident = np.eye(128, dtype=BF)

    if "nc" not in _CACHE:
        _CACHE["nc"] = _build_nc()
    nc = _CACHE["nc"]

    in_maps = [dict(fm=fm[i], gm=gm[i], w1pt=w1pt, bias=bias, w2et=w2et,
                    ident=ident)
               for i in range(B)]
    _CACHE["in_maps"] = in_maps
    res = bass_utils.run_bass_kernel_spmd(
        nc, in_maps, core_ids=list(range(B)), trace=TRACE)
    _CACHE["last"] = res
    out = np.stack([r["out"] for r in res.results], axis=0)
    return out.reshape(B, C, H, W)
